# revision 11
# baseline (speedup 1.0000x reference)
"""GCN layer kernel for Trainium2 (8 NeuronCores, Bass/Tile).

Computes: out = relu(rownorm(adj) @ (features @ W)) + eps
  features [N, F]  adj [N, N]  W [F, F]  ->  out [N, F]   (all fp32)

Strategy (row-sharded across 8 cores, no collectives):
  * Core c owns output rows [c*B, (c+1)*B), B = N/8.
  * Associativity restructure: out = relu(diag(1/rowsum) @ (adj_c @ F_aug) @ W)
    where F_aug = [features | 1 | 1].  This removes the baseline's redundant
    per-core "support = features @ W" pass over all N rows (~30us of PE time)
    and replaces it with a tiny per-row-tile epilogue (transpose + W matmul
    on 2048 rows only).
  * Main loop (per 128-row output tile, PSUM-accumulated over K=16384):
      psum[i] += adjT_tile.T @ F_aug[k]        (fp16 operands, fp32 psum)
    The ones column makes psum col F hold the adj row-sums.
  * Epilogue per tile (pipeline-interleaved into the next group's matmul
    stream so the PE never waits on the DVE):
      s1: copy psum -> SBUF fp32 (frees the accumulator bank fast)
      s2: rcp = 1/rowsum;  Tn = T * rcp  (cast fp16)
      s3: 2x PE transpose (Tn halves -> psum, fp16)
      s4: copy transposes -> SBUF fp16 (lhsT for the W matmul)
      s5: out_ps = Tt.T @ W  (2 matmuls, accumulated)
      s6: out = max(out_ps, 0) + eps  (fp32), DMA out
  * Row tiles processed in groups of (6,6,4) sharing a k-synchronous strip
    stream; group 0 is wide so the resident F_aug load (8.5 MB) spreads over
    enough PE time to fit under the HBM roofline alongside adj streaming.
  * dtype float16: PE streams 1 cycle/row with 2-byte DMA traffic; measured
    L2 rel err ~4e-4 for the baseline structure (same accumulation here).
"""

import sys

for _p in ("/opt/trn_rl_repo",):
    if _p not in sys.path:
        sys.path.append(_p)

import numpy as np

import concourse.bass as bass
import concourse.mybir as mybir
import concourse.tile as tile
from concourse import bacc
from concourse.bass_utils import run_bass_kernel_spmd
from concourse.masks import make_identity

N_TOTAL = 16384
F_DIM = 256
N_CORES = 8
BLOCK = N_TOTAL // N_CORES  # 2048 rows per core
EPS = 1e-4

DT_MAIN = mybir.dt.float16
KB = 2  # k-tiles packed per adjT strip DMA
FP = F_DIM + 2  # moving free dim: 256 features + ones col + pad
GROUPS = [(0, 6), (6, 6), (12, 4)]  # (first tile, n tiles): feat-load-heavy first


def build_nc(
    n_total: int = N_TOTAL,
    block: int = BLOCK,
    f: int = F_DIM,
    dt_main=DT_MAIN,
) -> bass.Bass:
    """Build the per-core Bass program (SPMD: same program, per-core data)."""
    assert n_total % 128 == 0 and block % 128 == 0 and f == 256
    kt_n = n_total // 128  # contraction tiles (128)
    it_n = block // 128  # output row tiles per core (16)
    assert kt_n % KB == 0

    nc = bacc.Bacc(None, target_bir_lowering=False)
    dt_f32 = mybir.dt.float32
    dt_sb = dt_main

    adjt_d = nc.declare_dram_parameter("adjt", [n_total * block], dt_sb, isOutput=False)
    featb_d = nc.declare_dram_parameter("featb", [kt_n * 128 * FP], dt_sb, isOutput=False)
    w_d = nc.declare_dram_parameter("w", [2 * 128 * f], dt_sb, isOutput=False)
    out_d = nc.declare_dram_parameter("out", [block, f], dt_f32, isOutput=True)

    with tile.TileContext(nc) as tc:
        with (
            tc.tile_pool(name="consts", bufs=1) as consts,
            tc.tile_pool(name="astr", bufs=24) as astr,
            tc.tile_pool(name="evac", bufs=6) as evac,
            tc.tile_pool(name="psM", bufs=6, space="PSUM") as psM,
            tc.tile_pool(name="psT", bufs=1, space="PSUM") as psT,
            tc.tile_pool(name="psO", bufs=1, space="PSUM") as psO,
        ):
            # ---- resident tensors
            # F_aug, k-tile major: per-partition 128*258*2 = 64.5 KB
            feat_sb = consts.tile([128, kt_n, FP], dt_sb, name="feat_sb", tag="feat_sb")
            w_sb = consts.tile([128, 2, f], dt_sb, name="w_sb", tag="w_sb")
            ident = consts.tile([128, 128], dt_sb, name="ident", tag="ident")

            def feat_dma(eng, c0, cn):
                src = featb_d[c0 * 128 * FP : (c0 + cn) * 128 * FP]
                src = src.rearrange("(t p w) -> p t w", t=cn, p=128)
                eng.dma_start(out=feat_sb[:, c0 : c0 + cn, :], in_=src)

            # Lead-in: first LEAD k-tiles of F_aug on the fast HWDGE queues
            # ahead of any strip (gates MM#0).  The rest is paced into the
            # strip stream (below) so it cannot race ahead of consumption
            # and starve the adj strips (HBM is ~95% subscribed in group 0).
            # All lead chunks go on the ACT queue so the first strips stream
            # back-to-back on the SYNC queue from t=0.
            LEAD = 12
            for c0 in range(0, LEAD, 2):
                feat_dma(nc.scalar, c0, 2)
            make_identity(nc, ident)
            # W needed only by the first epilogue (~90us in)
            for h in range(2):
                wsrc = w_d[h * 128 * f : (h + 1) * 128 * f]
                wsrc = wsrc.rearrange("(p w) -> p w", p=128)
                nc.gpsimd.dma_start(out=w_sb[:, h, :], in_=wsrc)

            # ---- epilogue stage machinery -----------------------------------
            def make_stages(it, pm, tail):
                """Return the list of stage closures for row tile `it`.
                `tail`: last group -> split DVE/ACT for the drain."""
                st = {}

                def s1():  # free the psum accumulator bank quickly
                    pmS = evac.tile([128, FP], dt_f32, name="pmS", tag="pmS")
                    nc.vector.tensor_copy(out=pmS, in_=pm)
                    st["pmS"] = pmS

                def s2():
                    pmS = st["pmS"]
                    rcp = evac.tile([128, 1], dt_f32, name="rcp", tag="rcp")
                    nc.vector.reciprocal(out=rcp, in_=pmS[:, f : f + 1])
                    tn = evac.tile([128, f], dt_sb, name="tn", tag="tn")
                    if tail:
                        # ACT: out = in * scale (per-partition scale AP)
                        nc.scalar.activation(
                            tn, pmS[:, 0:f], mybir.ActivationFunctionType.Copy,
                            scale=rcp,
                        )
                    else:
                        nc.vector.tensor_scalar_mul(tn, pmS[:, 0:f], rcp)
                    st["tn"] = tn

                def s3():
                    tn = st["tn"]
                    pt = psT.tile([128, 2, 128], dt_sb, name="pt", tag="pt")
                    nc.tensor.transpose(pt[:, 0, :], tn[:, 0:128], ident)
                    nc.tensor.transpose(pt[:, 1, :], tn[:, 128:256], ident)
                    st["pt"] = pt

                def s4():
                    pt = st["pt"]
                    tt = evac.tile([128, 2, 128], dt_sb, name="tt", tag="tt")
                    if tail:
                        nc.scalar.copy(out=tt[:, 0, :], in_=pt[:, 0, :])
                        nc.scalar.copy(out=tt[:, 1, :], in_=pt[:, 1, :])
                    else:
                        nc.vector.tensor_copy(out=tt[:, 0, :], in_=pt[:, 0, :])
                        nc.vector.tensor_copy(out=tt[:, 1, :], in_=pt[:, 1, :])
                    st["tt"] = tt

                def s5():
                    tt = st["tt"]
                    po = psO.tile([128, f], dt_f32, name="po", tag="po")
                    nc.tensor.matmul(po, lhsT=tt[:, 0, :], rhs=w_sb[:, 0, :],
                                     start=True, stop=False)
                    nc.tensor.matmul(po, lhsT=tt[:, 1, :], rhs=w_sb[:, 1, :],
                                     start=False, stop=True)
                    st["po"] = po

                def s6():
                    po = st["po"]
                    o = evac.tile([128, f], dt_f32, name="o", tag="o")
                    nc.vector.tensor_scalar(
                        out=o, in0=po, scalar1=0.0, scalar2=EPS,
                        op0=mybir.AluOpType.max, op1=mybir.AluOpType.add,
                    )
                    nc.gpsimd.dma_start(out=out_d[it * 128 : (it + 1) * 128, :], in_=o)

                return [s2, s3, s4, s5, s6], s1

            # ---- main loop: groups of row tiles, k-synchronous strip stream
            pending = []  # deferred stage closures from the previous group
            base = 0  # running offset into the packed adjt buffer
            ndma = 0
            for gi, (g0, gn) in enumerate(GROUPS):
                gw = gn * 128
                pms = [
                    psM.tile([128, FP], dt_f32, name=f"pm{j}", tag="pm")
                    for j in range(gn)
                ]
                for kb in range(kt_n // KB):
                    a = astr.tile([128, KB, 6 * 128], dt_sb, name="a", tag="a")
                    src = adjt_d[base + kb * KB * 128 * gw : base + (kb + 1) * KB * 128 * gw]
                    src = src.rearrange("(t p w) -> p t w", t=KB, p=128)
                    # first 4 strips back-to-back on SYNC (ACT is busy with
                    # the feat lead-in); alternate queues after that
                    if gi == 0 and kb < 4:
                        eng = nc.sync
                    else:
                        eng = nc.sync if ndma % 2 == 0 else nc.scalar
                        ndma += 1
                    eng.dma_start(out=a[:, :, 0:gw], in_=src)
                    if gi == 0:
                        # paced F_aug load: one 2-k-tile chunk behind each
                        # strip on the other queue, LEAD tiles ahead of use
                        c0 = kb * KB + LEAD
                        if c0 < kt_n:
                            feat_dma(nc.scalar if eng is nc.sync else nc.sync, c0, 2)
                    for t in range(KB):
                        k = kb * KB + t
                        for j in range(gn):
                            nc.tensor.matmul(
                                pms[j],
                                lhsT=a[:, t, j * 128 : (j + 1) * 128],
                                rhs=feat_sb[:, k, :],
                                start=(k == 0),
                                stop=(k == kt_n - 1),
                            )
                    # one deferred epilogue stage per k-chunk
                    if pending:
                        pending.pop(0)()
                base += kt_n * 128 * gw
                assert not pending, "stage backlog did not drain within a group"
                # boundary: free all accumulator banks first (s1 of each tile),
                # then queue the rest of the stages for interleaved emission.
                tail = gi == len(GROUPS) - 1
                s1s = []
                stageq = []
                for j in range(gn):
                    stages, s1 = make_stages(g0 + j, pms[j], tail)
                    s1s.append(s1)
                    stageq.append(stages)
                for s1 in s1s:
                    s1()
                if not tail:
                    for stages in stageq:
                        pending.extend(stages)
                else:
                    # drain: round-robin across tiles so engines pipeline
                    for si in range(5):
                        for stages in stageq:
                            stages[si]()

    nc.finalize()
    return nc


_NC_CACHE: dict = {}


def _get_nc(key=("v2",)):
    if key not in _NC_CACHE:
        _NC_CACHE[key] = build_nc()
    return _NC_CACHE[key]


def pack_adjt(adj_rows: np.ndarray, n_total: int, np_dt=np.float16) -> np.ndarray:
    """Pack a [block, n_total] row-slab of adj into the strip-major layout the
    kernel streams: per tile group g, per k-tile: a contiguous [128, gw]
    brick of adjT (k on rows)."""
    kt_n = n_total // 128
    out = np.empty(adj_rows.size, dtype=np_dt)
    pos = 0
    for g0, gn in GROUPS:
        gw = gn * 128
        sub = adj_rows[g0 * 128 : g0 * 128 + gw, :]  # [gw, n_total]
        # adjT[k, i] tiled -> [kt_n, 128, gw]
        brick = sub.reshape(gw, kt_n, 128).transpose(1, 2, 0)
        n = brick.size
        out[pos : pos + n] = brick.reshape(-1)
        pos += n
    return out


def make_in_maps(features: np.ndarray, adj: np.ndarray, weight: np.ndarray,
                 dt_main=DT_MAIN):
    np_dt = np.float16
    kt_n = N_TOTAL // 128
    # F_aug = [features | 1 | 1], k-tile-major bricks [kt, 128, FP]
    featb = np.ones((kt_n, 128, FP), dtype=np_dt)
    featb[:, :, 0:F_DIM] = (
        np.asarray(features, dtype=np.float32)
        .astype(np_dt, copy=False)
        .reshape(kt_n, 128, F_DIM)
    )
    featb = featb.reshape(-1)
    w = (
        np.ascontiguousarray(np.asarray(weight, dtype=np.float32))
        .astype(np_dt, copy=False)
        .reshape(-1)
    )
    adj = np.asarray(adj, dtype=np.float32).astype(np_dt, copy=False)
    in_maps = []
    for c in range(N_CORES):
        adjt_c = pack_adjt(adj[c * BLOCK : (c + 1) * BLOCK, :], N_TOTAL, np_dt)
        in_maps.append({"adjt": adjt_c, "featb": featb, "w": w})
    return in_maps


def _run_once(nc, in_maps):
    last_err = None
    for attempt in range(3):
        try:
            res = run_bass_kernel_spmd(nc, in_maps, core_ids=list(range(N_CORES)))
            return np.concatenate(
                [res.results[c]["out"] for c in range(N_CORES)], axis=0
            )
        except Exception as e:  # transient NRT/device hiccups: back off, retry
            last_err = e
            import time
            time.sleep(20 * (attempt + 1))
    raise last_err


def kernel(features: np.ndarray, adj: np.ndarray, weight: np.ndarray) -> np.ndarray:
    nc = _get_nc()
    in_maps = make_in_maps(features, adj, weight)
    # Run twice and compare: guards against rare transient silent corruption
    # on a cold device (observed once: first exec returned garbage).
    o1 = _run_once(nc, in_maps)
    o2 = _run_once(nc, in_maps)
    if np.array_equal(o1, o2):
        return o1
    o3 = _run_once(nc, in_maps)
    if np.array_equal(o1, o3) or np.array_equal(o2, o3):
        return o3
    return o1


if __name__ == "__main__":
    rng = np.random.default_rng(0)
    feats = rng.standard_normal((N_TOTAL, F_DIM), dtype=np.float32)
    adj = rng.random((N_TOTAL, N_TOTAL), dtype=np.float32)
    w = rng.standard_normal((F_DIM, F_DIM), dtype=np.float32) * 0.06
    out = kernel(feats, adj, w)
    # numpy check
    adjn = adj / adj.sum(axis=1, keepdims=True)
    exp = np.maximum(adjn @ (feats @ w), 0.0) + EPS
    err = np.linalg.norm(out - exp) / np.linalg.norm(exp)
    print(out.shape, out.dtype, "rel_err", err)


# revision 14
# speedup vs baseline: 1.0722x; 1.0722x over previous
"""GCN layer kernel for Trainium2 (8 NeuronCores, Bass/Tile).

Computes: out = relu(rownorm(adj) @ (features @ W)) + eps
  features [N, F]  adj [N, N]  W [F, F]  ->  out [N, F]   (all fp32)

Strategy (row-sharded across 8 cores, no collectives):
  * Core c owns output rows [c*B, (c+1)*B), B = N/8.
  * Associativity restructure: out = relu(diag(1/rowsum) @ (adj_c @ F_aug) @ W)
    where F_aug = [features | 1 | 1].  This removes the baseline's redundant
    per-core "support = features @ W" pass over all N rows (~30us of PE time)
    and replaces it with a tiny per-row-tile epilogue (transpose + W matmul
    on 2048 rows only).
  * Main loop (per 128-row output tile, PSUM-accumulated over K=16384):
      psum[i] += adjT_tile.T @ F_aug[k]        (fp16 operands, fp32 psum)
    The ones column makes psum col F hold the adj row-sums.
  * Epilogue per tile (pipeline-interleaved into the next group's matmul
    stream so the PE never waits on the DVE):
      s1: copy psum -> SBUF fp32 (frees the accumulator bank fast)
      s2: rcp = 1/rowsum;  Tn = T * rcp  (cast fp16)
      s3: 2x PE transpose (Tn halves -> psum, fp16)
      s4: copy transposes -> SBUF fp16 (lhsT for the W matmul)
      s5: out_ps = Tt.T @ W  (2 matmuls, accumulated)
      s6: out = max(out_ps, 0) + eps  (fp32), DMA out
  * Row tiles processed in groups of (6,6,4) sharing a k-synchronous strip
    stream; group 0 is wide so the resident F_aug load (8.5 MB) spreads over
    enough PE time to fit under the HBM roofline alongside adj streaming.
  * dtype float16: PE streams 1 cycle/row with 2-byte DMA traffic; measured
    L2 rel err ~4e-4 for the baseline structure (same accumulation here).
"""

import sys

for _p in ("/opt/trn_rl_repo",):
    if _p not in sys.path:
        sys.path.append(_p)

import numpy as np

import concourse.bass as bass
import concourse.mybir as mybir
import concourse.tile as tile
from concourse import bacc
from concourse.bass_utils import run_bass_kernel_spmd
from concourse.masks import make_identity

N_TOTAL = 16384
F_DIM = 256
N_CORES = 8
BLOCK = N_TOTAL // N_CORES  # 2048 rows per core
EPS = 1e-4

DT_MAIN = mybir.dt.float16
KB = 2  # k-tiles packed per adjT strip DMA
FP = F_DIM + 2  # moving free dim: 256 features + ones col + pad
GROUPS = [(0, 6), (6, 6), (12, 4)]  # (first tile, n tiles): feat-load-heavy first


def build_nc(
    n_total: int = N_TOTAL,
    block: int = BLOCK,
    f: int = F_DIM,
    dt_main=DT_MAIN,
) -> bass.Bass:
    """Build the per-core Bass program (SPMD: same program, per-core data)."""
    assert n_total % 128 == 0 and block % 128 == 0 and f == 256
    kt_n = n_total // 128  # contraction tiles (128)
    it_n = block // 128  # output row tiles per core (16)
    assert kt_n % KB == 0

    nc = bacc.Bacc(None, target_bir_lowering=False)
    dt_f32 = mybir.dt.float32
    dt_sb = dt_main

    adjt_d = nc.declare_dram_parameter("adjt", [n_total * block], dt_sb, isOutput=False)
    featb_d = nc.declare_dram_parameter("featb", [kt_n * 128 * FP], dt_sb, isOutput=False)
    w_d = nc.declare_dram_parameter("w", [2 * 128 * f], dt_sb, isOutput=False)
    out_d = nc.declare_dram_parameter("out", [block, f], dt_f32, isOutput=True)

    with tile.TileContext(nc) as tc:
        with (
            tc.tile_pool(name="consts", bufs=1) as consts,
            tc.tile_pool(name="astr", bufs=16) as astr,
            tc.tile_pool(name="evac", bufs=6) as evac,
            tc.tile_pool(name="psM", bufs=6, space="PSUM") as psM,
            tc.tile_pool(name="psT", bufs=1, space="PSUM") as psT,
            tc.tile_pool(name="psO", bufs=1, space="PSUM") as psO,
        ):
            # ---- resident tensors
            # F_aug, k-tile major: per-partition 128*258*2 = 64.5 KB
            feat_sb = consts.tile([128, kt_n, FP], dt_sb, name="feat_sb", tag="feat_sb")
            w_sb = consts.tile([128, 2, f], dt_sb, name="w_sb", tag="w_sb")
            ident = consts.tile([128, 128], dt_sb, name="ident", tag="ident")

            def feat_dma(eng, c0, cn):
                src = featb_d[c0 * 128 * FP : (c0 + cn) * 128 * FP]
                src = src.rearrange("(t p w) -> p t w", t=cn, p=128)
                eng.dma_start(out=feat_sb[:, c0 : c0 + cn, :], in_=src)

            # Lead-in: first LEAD k-tiles of F_aug on the fast HWDGE queues
            # ahead of any strip (gates MM#0).  The rest is paced into the
            # strip stream (below) so it cannot race ahead of consumption
            # and starve the adj strips (HBM is ~95% subscribed in group 0).
            # F_aug rides the GPSIMD queue (3rd DMA queue — the two HWDGE
            # queues saturate at ~175 GB/s each and group 0 needs them for
            # strips).  Lead chunks issue immediately; the rest are paced by
            # a marker copy that waits for the matching strip to land, so
            # feat cannot race ahead and oversubscribe HBM (group 0 runs at
            # ~95% of the HBM roofline).
            LEAD = 12
            for c0 in range(0, LEAD, 2):
                feat_dma(nc.gpsimd, c0, 2)
            # W needed only by the first epilogue (~90us in)
            for h in range(2):
                wsrc = w_d[h * 128 * f : (h + 1) * 128 * f]
                wsrc = wsrc.rearrange("(p w) -> p w", p=128)
                nc.gpsimd.dma_start(out=w_sb[:, h, :], in_=wsrc)
            make_identity(nc, ident)
            marker = consts.tile([128, 1], dt_sb, name="marker", tag="marker")

            # ---- epilogue stage machinery -----------------------------------
            def make_stages(it, pm, tail):
                """Return the list of stage closures for row tile `it`.
                `tail`: last group -> split DVE/ACT for the drain."""
                st = {}

                def s1():  # free the psum accumulator bank quickly
                    pmS = evac.tile([128, FP], dt_f32, name="pmS", tag="pmS")
                    nc.vector.tensor_copy(out=pmS, in_=pm)
                    st["pmS"] = pmS

                def s2():
                    pmS = st["pmS"]
                    rcp = evac.tile([128, 1], dt_f32, name="rcp", tag="rcp")
                    nc.vector.reciprocal(out=rcp, in_=pmS[:, f : f + 1])
                    tn = evac.tile([128, f], dt_sb, name="tn", tag="tn")
                    if tail:
                        # ACT: out = in * scale (per-partition scale AP)
                        nc.scalar.activation(
                            tn, pmS[:, 0:f], mybir.ActivationFunctionType.Copy,
                            scale=rcp,
                        )
                    else:
                        nc.vector.tensor_scalar_mul(tn, pmS[:, 0:f], rcp)
                    st["tn"] = tn

                def s3():
                    tn = st["tn"]
                    pt = psT.tile([128, 2, 128], dt_sb, name="pt", tag="pt")
                    nc.tensor.transpose(pt[:, 0, :], tn[:, 0:128], ident)
                    nc.tensor.transpose(pt[:, 1, :], tn[:, 128:256], ident)
                    st["pt"] = pt

                def s4():
                    pt = st["pt"]
                    tt = evac.tile([128, 2, 128], dt_sb, name="tt", tag="tt")
                    if tail:
                        nc.scalar.copy(out=tt[:, 0, :], in_=pt[:, 0, :])
                        nc.scalar.copy(out=tt[:, 1, :], in_=pt[:, 1, :])
                    else:
                        nc.vector.tensor_copy(out=tt[:, 0, :], in_=pt[:, 0, :])
                        nc.vector.tensor_copy(out=tt[:, 1, :], in_=pt[:, 1, :])
                    st["tt"] = tt

                def s5():
                    tt = st["tt"]
                    po = psO.tile([128, f], dt_f32, name="po", tag="po")
                    nc.tensor.matmul(po, lhsT=tt[:, 0, :], rhs=w_sb[:, 0, :],
                                     start=True, stop=False)
                    nc.tensor.matmul(po, lhsT=tt[:, 1, :], rhs=w_sb[:, 1, :],
                                     start=False, stop=True)
                    st["po"] = po

                def s6():
                    po = st["po"]
                    o = evac.tile([128, f], dt_f32, name="o", tag="o")
                    nc.vector.tensor_scalar(
                        out=o, in0=po, scalar1=0.0, scalar2=EPS,
                        op0=mybir.AluOpType.max, op1=mybir.AluOpType.add,
                    )
                    nc.gpsimd.dma_start(out=out_d[it * 128 : (it + 1) * 128, :], in_=o)

                return [s2, s3, s4, s5, s6], s1

            # ---- main loop: groups of row tiles, k-synchronous strip stream
            pending = []  # deferred stage closures from the previous group
            base = 0  # running offset into the packed adjt buffer
            ndma = 0
            for gi, (g0, gn) in enumerate(GROUPS):
                gw = gn * 128
                pms = [
                    psM.tile([128, FP], dt_f32, name=f"pm{j}", tag="pm")
                    for j in range(gn)
                ]
                for kb in range(kt_n // KB):
                    a = astr.tile([128, KB, 6 * 128], dt_sb, name="a", tag="a")
                    src = adjt_d[base + kb * KB * 128 * gw : base + (kb + 1) * KB * 128 * gw]
                    src = src.rearrange("(t p w) -> p t w", t=KB, p=128)
                    eng = nc.sync if ndma % 2 == 0 else nc.scalar
                    ndma += 1
                    eng.dma_start(out=a[:, :, 0:gw], in_=src)
                    if gi == 0:
                        c0 = kb * KB + LEAD
                        if c0 < kt_n:
                            # pace: wait for this strip to land, then fetch
                            # the feat chunk LEAD tiles ahead of consumption
                            nc.gpsimd.tensor_copy(out=marker, in_=a[:, 0, 0:1])
                            feat_dma(nc.gpsimd, c0, 2)
                    for t in range(KB):
                        k = kb * KB + t
                        for j in range(gn):
                            nc.tensor.matmul(
                                pms[j],
                                lhsT=a[:, t, j * 128 : (j + 1) * 128],
                                rhs=feat_sb[:, k, :],
                                start=(k == 0),
                                stop=(k == kt_n - 1),
                            )
                    # one deferred epilogue stage per k-chunk
                    if pending:
                        pending.pop(0)()
                base += kt_n * 128 * gw
                assert not pending, "stage backlog did not drain within a group"
                # boundary: free all accumulator banks first (s1 of each tile),
                # then queue the rest of the stages for interleaved emission.
                tail = gi == len(GROUPS) - 1
                s1s = []
                stageq = []
                for j in range(gn):
                    stages, s1 = make_stages(g0 + j, pms[j], tail)
                    s1s.append(s1)
                    stageq.append(stages)
                for s1 in s1s:
                    s1()
                if not tail:
                    for stages in stageq:
                        pending.extend(stages)
                else:
                    # drain: round-robin across tiles so engines pipeline
                    for si in range(5):
                        for stages in stageq:
                            stages[si]()

    nc.finalize()
    return nc


_NC_CACHE: dict = {}


def _get_nc(key=("v2",)):
    if key not in _NC_CACHE:
        _NC_CACHE[key] = build_nc()
    return _NC_CACHE[key]


def pack_adjt(adj_rows: np.ndarray, n_total: int, np_dt=np.float16) -> np.ndarray:
    """Pack a [block, n_total] row-slab of adj into the strip-major layout the
    kernel streams: per tile group g, per k-tile: a contiguous [128, gw]
    brick of adjT (k on rows)."""
    kt_n = n_total // 128
    out = np.empty(adj_rows.size, dtype=np_dt)
    pos = 0
    for g0, gn in GROUPS:
        gw = gn * 128
        sub = adj_rows[g0 * 128 : g0 * 128 + gw, :]  # [gw, n_total]
        # adjT[k, i] tiled -> [kt_n, 128, gw]
        brick = sub.reshape(gw, kt_n, 128).transpose(1, 2, 0)
        n = brick.size
        out[pos : pos + n] = brick.reshape(-1)
        pos += n
    return out


def make_in_maps(features: np.ndarray, adj: np.ndarray, weight: np.ndarray,
                 dt_main=DT_MAIN):
    np_dt = np.float16
    kt_n = N_TOTAL // 128
    # F_aug = [features | 1 | 1], k-tile-major bricks [kt, 128, FP]
    featb = np.ones((kt_n, 128, FP), dtype=np_dt)
    featb[:, :, 0:F_DIM] = (
        np.asarray(features, dtype=np.float32)
        .astype(np_dt, copy=False)
        .reshape(kt_n, 128, F_DIM)
    )
    featb = featb.reshape(-1)
    w = (
        np.ascontiguousarray(np.asarray(weight, dtype=np.float32))
        .astype(np_dt, copy=False)
        .reshape(-1)
    )
    adj = np.asarray(adj, dtype=np.float32).astype(np_dt, copy=False)
    in_maps = []
    for c in range(N_CORES):
        adjt_c = pack_adjt(adj[c * BLOCK : (c + 1) * BLOCK, :], N_TOTAL, np_dt)
        in_maps.append({"adjt": adjt_c, "featb": featb, "w": w})
    return in_maps


def _run_once(nc, in_maps):
    last_err = None
    for attempt in range(3):
        try:
            res = run_bass_kernel_spmd(nc, in_maps, core_ids=list(range(N_CORES)))
            return np.concatenate(
                [res.results[c]["out"] for c in range(N_CORES)], axis=0
            )
        except Exception as e:  # transient NRT/device hiccups: back off, retry
            last_err = e
            import time
            time.sleep(20 * (attempt + 1))
    raise last_err


def kernel(features: np.ndarray, adj: np.ndarray, weight: np.ndarray) -> np.ndarray:
    nc = _get_nc()
    in_maps = make_in_maps(features, adj, weight)
    # Run twice and compare: guards against rare transient silent corruption
    # on a cold device (observed once: first exec returned garbage).
    o1 = _run_once(nc, in_maps)
    o2 = _run_once(nc, in_maps)
    if np.array_equal(o1, o2):
        return o1
    o3 = _run_once(nc, in_maps)
    if np.array_equal(o1, o3) or np.array_equal(o2, o3):
        return o3
    return o1


if __name__ == "__main__":
    rng = np.random.default_rng(0)
    feats = rng.standard_normal((N_TOTAL, F_DIM), dtype=np.float32)
    adj = rng.random((N_TOTAL, N_TOTAL), dtype=np.float32)
    w = rng.standard_normal((F_DIM, F_DIM), dtype=np.float32) * 0.06
    out = kernel(feats, adj, w)
    # numpy check
    adjn = adj / adj.sum(axis=1, keepdims=True)
    exp = np.maximum(adjn @ (feats @ w), 0.0) + EPS
    err = np.linalg.norm(out - exp) / np.linalg.norm(exp)
    print(out.shape, out.dtype, "rel_err", err)
